# revision 44
# baseline (speedup 1.0000x reference)
"""Sliding-window (banded) multi-head self-attention on 8 trn2 NeuronCores.

Sequence-parallel sharding: batch b, 2048 tokens -> 4 chunks of 512 queries;
core c handles batch c//4, chunk c%4.  Each core receives x^T for its 512
tokens plus a 128-token halo (zero-padded for chunk 0), computes
qkv projection + RoPE + banded attention (window 129) + out projection for
its rows, and returns [512, 2048].  No cross-core communication.

v2 (vs. baseline): everything bf16 (halves DMA bytes and PE weight-load
time), single large DMA per weight tensor, per-ec x tiles for fine-grained
startup deps, RoPE pairs laid out 16-apart within each 32-partition quadrant
so rotate_half is ONE DVE stream_shuffle (no SBUF->SBUF DMA), mask-multiply
moved to GpSimd, reciprocal via approx-fast, and a lookahead-2 software
pipeline that interleaves head h's attention matmuls with head h+2's
projection matmuls so the PE never waits on the RoPE/exp chains.

Layout choices (all matmuls contract over the partition dim):
  - x^T resident in SBUF as 16 tiles [128, 640(tok)] bf16 (one per e-chunk)
  - Q^T/K^T per head feature-major [128(d), tok] from PSUM; RoPE pairs
    de-interleaved host-side per 32-partition quadrant (16 evens, 16 odds)
    so rotate_half is stream_shuffle(mask=[16..31,0..15]).
  - V token-major [128(tok), d] (natural for PV lhsT).
  - scores^T per k-chunk as [128(k), 256(q)] bf16 matmuls; exp on ACT;
    0/1 band-mask multiply on GpSimd; PV + replicated-ones rowsum matmuls
    accumulate into one PSUM bank; normalize with reciprocal_approx_fast.
  - out projection accumulates 16 hd-chunks; bias added via K=1 ones matmul.
"""

import math
import numpy as np
import ml_dtypes

import concourse.bass as bass
import concourse.tile as tile
from concourse import mybir
from concourse.bass_utils import run_bass_kernel_spmd
from concourse.vector_clock import ScopedClock, VectorClock


def _legalize_single_wait(nc):
    """This walrus build accepts only ONE sync-wait per lowered command
    ("Too many sync wait commands").  Move all but the last wait of every
    instruction onto single-wait NoOps prepended on the same engine: engines
    are in-order, so stalling on the NoOps is equivalent.  Engine-issued DMAs
    are gated the same way (descriptor push happens in program order)."""
    nid = [0]
    for f in nc.m.functions:
        for blk in f.blocks:
            out = []
            changed = False
            for inst in blk.instructions:
                si = inst.sync_info
                waits = list(si.on_wait) if si and si.on_wait else []
                if len(waits) > 1:
                    changed = True
                    for w in waits[:-1]:
                        nop = mybir.InstNoOp(name=f"waitnop-{nid[0]}", ins=[], outs=[])
                        nid[0] += 1
                        nop.engine = inst.engine
                        nop.sync_info = mybir.SyncInfo(on_wait=[w], on_update=[])
                        out.append(nop)
                    inst.sync_info = mybir.SyncInfo(
                        on_wait=[waits[-1]], on_update=list(si.on_update or [])
                    )
                out.append(inst)
            if changed:
                blk.instructions = out
    return nc


def _install_drain_split_patch():
    """Split TileContext's closing drain into single-wait drains: walrus's
    CTRL_NO command rejects the catch-all drain ("Too many sync waits")."""
    if getattr(tile.TileContext, "_drain_split_patched", False):
        return

    def _patched(self, tick_clock, wait_clock):
        gvc = tick_clock.global_clock  # VectorClock over the 27 procs
        n = len(gvc)
        procs = [i for i in range(n) if gvc[i] > 0]
        for pi in procs:
            vc = VectorClock([gvc[i] if i == pi else 0 for i in range(n)])
            d = self.nc.sync.drain()
            wait_clock.add_sem_waits(d.ins, ScopedClock({None: vc}))
        self.nc.all_engine_barrier()
        assert self.sems is not None
        popped = self.nc._tile_sem_poison_stack.pop()
        assert popped is self._sem_poison
        self.nc.clear_and_free_semaphores(list(self.sems.allocated().values()))
        self.nc.all_engine_barrier()

    tile.TileContext._drain_and_barrier = _patched
    tile.TileContext._drain_split_patched = True


_install_drain_split_patch()

EMBED = 2048
HEADS = 16
HD = 128
WINDOW = 128
THETA = 10000.0
B = 2
L = 2048
S = 512            # queries per core
T = S + WINDOW     # k/v tokens per core (incl halo)
NCORES = 8
P = 128
F32 = mybir.dt.float32
BF16 = mybir.dt.bfloat16

EC = EMBED // P    # 16 e-chunks
GROUPS = 4         # head groups of 4 (for V projection at N=512)
GH = HEADS // GROUPS
HT = T // 2        # 320

# rotate_half as a within-quadrant shuffle: out[32s+i] = in[32s+mask[i]]
SHUF = list(range(16, 32)) + list(range(0, 16))


def build_bass(legalize=True):
    nc = bass.Bass("TRN2", target_bir_lowering=False, debug=False)

    XT = nc.dram_tensor("XT", [EC, P, T], BF16, kind="ExternalInput")
    WQ = nc.dram_tensor("WQ", [HEADS, P, EC, HD], BF16, kind="ExternalInput")
    WK = nc.dram_tensor("WK", [HEADS, P, EC, HD], BF16, kind="ExternalInput")
    WV = nc.dram_tensor("WV", [GROUPS, P, EC, 512], BF16, kind="ExternalInput")
    WO = nc.dram_tensor("WO", [4, P, EC, 512], BF16, kind="ExternalInput")
    BIASB = nc.dram_tensor("BIASB", [P, EMBED], BF16, kind="ExternalInput")
    COSK = nc.dram_tensor("COSK", [P, T], BF16, kind="ExternalInput")
    SINK = nc.dram_tensor("SINK", [P, T], BF16, kind="ExternalInput")
    MASKS = nc.dram_tensor("MASKS", [P, 4, 256], BF16, kind="ExternalInput")
    ONES = nc.dram_tensor("ONES", [P, P], BF16, kind="ExternalInput")
    OUT = nc.dram_tensor("OUT", [S, EMBED], F32, kind="ExternalOutput")

    with tile.TileContext(nc) as tc:
        with (
            tc.tile_pool(name="persist", bufs=1) as persist,
            tc.tile_pool(name="wqk", bufs=8) as wqk,
            tc.tile_pool(name="wvp", bufs=2) as wvp,
            tc.tile_pool(name="wop", bufs=2) as wop,
            tc.tile_pool(name="rope", bufs=8) as rope,
            tc.tile_pool(name="qk", bufs=8) as qk,
            tc.tile_pool(name="vsb", bufs=10) as vsb,
            tc.tile_pool(name="etp", bufs=6) as etp,
            tc.tile_pool(name="denp", bufs=4) as denp,
            tc.tile_pool(name="outsb", bufs=2) as outsb,
            tc.tile_pool(name="ps_big", bufs=3, space="PSUM") as ps_big,
            tc.tile_pool(name="ps_k", bufs=2, space="PSUM") as ps_k,
            tc.tile_pool(name="ps_sc", bufs=3, space="PSUM") as ps_sc,
        ):
            # ---- persistent tiles ----
            xt = [
                persist.tile([P, T], BF16, tag=f"xt{ec}", name=f"xt{ec}")
                for ec in range(EC)
            ]
            cosk = persist.tile([P, T], BF16, tag="cosk")
            sink = persist.tile([P, T], BF16, tag="sink")
            cosq = cosk[:, WINDOW:T]
            sinq = sink[:, WINDOW:T]
            masks = persist.tile([P, 4, 256], BF16, tag="masks")
            ones = persist.tile([P, P], BF16, tag="ones")
            biasb = persist.tile([P, EMBED], BF16, tag="biasb")
            out_norm = persist.tile([P, HEADS, S], BF16, tag="out_norm")

            wq_tiles, wk_tiles, wv_tiles, wo_tiles = {}, {}, {}, {}
            q_tiles, k_tiles, v_groups, ets, pocs = {}, {}, {}, {}, {}

            def load_qk(h, eng=None):
                eng = eng or nc.scalar
                wq = wqk.tile([P, EC, HD], BF16, tag="wqk", name=f"wq{h}")
                eng.dma_start(wq, WQ.ap()[h])
                wk = wqk.tile([P, EC, HD], BF16, tag="wqk", name=f"wk{h}")
                eng.dma_start(wk, WK.ap()[h])
                wq_tiles[h] = wq
                wk_tiles[h] = wk

            def load_wv(g):
                wv = wvp.tile([P, EC, 512], BF16, tag="wv", name=f"wv{g}")
                nc.sync.dma_start(wv, WV.ap()[g])
                wv_tiles[g] = wv

            def load_wo(eo):
                wo = wop.tile([P, EC, 512], BF16, tag="wo", name=f"wo{eo}")
                nc.sync.dma_start(wo, WO.ap()[eo])
                wo_tiles[eo] = wo

            def emit_qproj(h):
                wq = wq_tiles[h]
                psq = ps_big.tile([P, S], F32, tag="big", name=f"psq{h}")
                for ec in range(EC):
                    nc.tensor.matmul(
                        psq,
                        wq[:, ec, :],
                        xt[ec][:, WINDOW:T],
                        start=(ec == 0),
                        stop=(ec == EC - 1),
                    )
                qsw = rope.tile([P, S], F32, tag="rope", name=f"qsw{h}")
                nc.vector.stream_shuffle(qsw, psq, SHUF)
                nc.gpsimd.tensor_mul(qsw, qsw, sinq)
                qc = rope.tile([P, S], F32, tag="rope", name=f"qc{h}")
                nc.vector.tensor_mul(qc, psq, cosq)
                q_sb = qk.tile([P, S], BF16, tag="qk", name=f"q{h}")
                nc.vector.tensor_add(q_sb, qc, qsw)
                q_tiles[h] = q_sb

            def emit_proj_interleaved(h):
                # startup variant: Q/K matmuls in 4-ec blocks so the PE
                # tracks the streaming x arrival instead of stalling per pass
                wq, wk = wq_tiles[h], wk_tiles[h]
                psq = ps_big.tile([P, S], F32, tag="big", name=f"psq{h}")
                psk1 = ps_k.tile([P, HT], F32, tag="k", name=f"psk1_{h}")
                psk2 = ps_k.tile([P, HT], F32, tag="k", name=f"psk2_{h}")
                for blk in range(4):
                    for ec in range(4 * blk, 4 * blk + 4):
                        nc.tensor.matmul(
                            psq, wq[:, ec, :], xt[ec][:, WINDOW:T],
                            start=(ec == 0), stop=(ec == EC - 1),
                        )
                    for ec in range(4 * blk, 4 * blk + 4):
                        nc.tensor.matmul(
                            psk1, wk[:, ec, :], xt[ec][:, 0:HT],
                            start=(ec == 0), stop=(ec == EC - 1),
                        )
                    for ec in range(4 * blk, 4 * blk + 4):
                        nc.tensor.matmul(
                            psk2, wk[:, ec, :], xt[ec][:, HT:T],
                            start=(ec == 0), stop=(ec == EC - 1),
                        )
                qsw = rope.tile([P, S], F32, tag="rope", name=f"qsw{h}")
                nc.vector.stream_shuffle(qsw, psq, SHUF)
                nc.gpsimd.tensor_mul(qsw, qsw, sinq)
                qc = rope.tile([P, S], F32, tag="rope", name=f"qc{h}")
                nc.vector.tensor_mul(qc, psq, cosq)
                q_sb = qk.tile([P, S], BF16, tag="qk", name=f"q{h}")
                nc.vector.tensor_add(q_sb, qc, qsw)
                q_tiles[h] = q_sb
                ksw = rope.tile([P, T], F32, tag="rope", name=f"ksw{h}")
                nc.vector.stream_shuffle(ksw[:, 0:HT], psk1, SHUF)
                nc.vector.stream_shuffle(ksw[:, HT:T], psk2, SHUF)
                nc.gpsimd.tensor_mul(ksw, ksw, sink)
                kc = rope.tile([P, T], F32, tag="rope", name=f"kc{h}")
                nc.vector.tensor_mul(kc[:, 0:HT], psk1, cosk[:, 0:HT])
                nc.vector.tensor_mul(kc[:, HT:T], psk2, cosk[:, HT:T])
                k_sb = qk.tile([P, T], BF16, tag="qk", name=f"k{h}")
                nc.vector.tensor_add(k_sb, kc, ksw)
                k_tiles[h] = k_sb

            def emit_kproj(h):
                wk = wk_tiles[h]
                psk1 = ps_k.tile([P, HT], F32, tag="k", name=f"psk1_{h}")
                psk2 = ps_k.tile([P, HT], F32, tag="k", name=f"psk2_{h}")
                for ec in range(EC):
                    nc.tensor.matmul(
                        psk1,
                        wk[:, ec, :],
                        xt[ec][:, 0:HT],
                        start=(ec == 0),
                        stop=(ec == EC - 1),
                    )
                for ec in range(EC):
                    nc.tensor.matmul(
                        psk2,
                        wk[:, ec, :],
                        xt[ec][:, HT:T],
                        start=(ec == 0),
                        stop=(ec == EC - 1),
                    )
                ksw = rope.tile([P, T], F32, tag="rope", name=f"ksw{h}")
                nc.vector.stream_shuffle(ksw[:, 0:HT], psk1, SHUF)
                nc.vector.stream_shuffle(ksw[:, HT:T], psk2, SHUF)
                nc.gpsimd.tensor_mul(ksw, ksw, sink)
                kc = rope.tile([P, T], F32, tag="rope", name=f"kc{h}")
                nc.vector.tensor_mul(kc[:, 0:HT], psk1, cosk[:, 0:HT])
                nc.vector.tensor_mul(kc[:, HT:T], psk2, cosk[:, HT:T])
                k_sb = qk.tile([P, T], BF16, tag="qk", name=f"k{h}")
                nc.vector.tensor_add(k_sb, kc, ksw)
                k_tiles[h] = k_sb

            def emit_vproj(g):
                wv = wv_tiles[g]
                vts = []
                for tt in range(T // P):
                    psv = ps_big.tile([P, 512], F32, tag="big", name=f"psv{g}_{tt}")
                    for ec in range(EC):
                        nc.tensor.matmul(
                            psv,
                            xt[ec][:, tt * P : (tt + 1) * P],
                            wv[:, ec, :],
                            start=(ec == 0),
                            stop=(ec == EC - 1),
                        )
                    v_t = vsb.tile([P, 512], BF16, tag="v", name=f"v{g}_{tt}")
                    nc.scalar.copy(v_t, psv)
                    vts.append(v_t)
                v_groups[g] = vts

            def emit_scores_half(h, p):
                qs = p * 256
                for j in range(3):
                    c = 2 * p + j
                    midx = 3 if (j == 0 and p == 1) else j
                    psc = ps_sc.tile([P, 256], F32, tag="sc", name=f"sc{h}_{p}{j}")
                    nc.tensor.matmul(
                        psc,
                        k_tiles[h][:, c * P : (c + 1) * P],
                        q_tiles[h][:, qs : qs + 256],
                        start=True,
                        stop=True,
                    )
                    et = etp.tile([P, 256], BF16, tag="et", name=f"et{h}_{p}{j}")
                    nc.scalar.activation(
                        et, psc, mybir.ActivationFunctionType.Exp,
                        scale=1.0 / math.sqrt(HD),
                    )
                    nc.gpsimd.tensor_mul(et, et, masks[:, midx, :])
                    ets[(h, p, j)] = et

            def emit_pv_half(h, p):
                hh = h % GH
                vts = v_groups[h // GH]
                poc = ps_big.tile([P, 512], F32, tag="big", name=f"poc{h}_{p}")
                ets3 = [ets.pop((h, p, j)) for j in range(3)]
                for j in range(3):
                    nc.tensor.matmul(
                        poc[:, 0:256],
                        vts[2 * p + j][:, hh * HD : (hh + 1) * HD],
                        ets3[j],
                        start=(j == 0),
                        stop=False,
                    )
                for j in range(3):
                    nc.tensor.matmul(
                        poc[:, 256:512],
                        ones,
                        ets3[j],
                        start=False,
                        stop=(j == 2),
                    )
                pocs[(h, p)] = poc

            def emit_recip_norm(h):
                for p in range(2):
                    poc = pocs.pop((h, p))
                    rc = denp.tile([P, 256], F32, tag="rc", name=f"rc{h}_{p}")
                    nc.vector.reciprocal(rc, poc[:, 256:512])
                    nc.vector.tensor_mul(
                        out_norm[:, h, p * 256 : (p + 1) * 256], poc[:, 0:256], rc
                    )

            # ---- prologue: loads + heads 0,1 projection + V group 0 ----
            # p-state warmup: dummy matmuls on memset data keep the PE busy
            # from ~8us so the clock is ramped when real work arrives ~12us
            warm = persist.tile([P, 512], BF16, tag="warm")
            nc.gpsimd.memset(warm, 0.0)
            for i in range(22):
                psw = ps_sc.tile([P, 256], F32, tag="sc", name=f"warm{i}")
                nc.tensor.matmul(
                    psw, warm[:, 0:P], warm[:, 256:512], start=True, stop=True
                )

            load_qk(0)
            for ec in range(EC // 2):
                nc.sync.dma_start(xt[ec], XT.ap()[ec])
            for ec in range(EC // 2, EC):
                nc.scalar.dma_start(xt[ec], XT.ap()[ec])
            load_qk(1)
            nc.gpsimd.dma_start(cosk, COSK.ap())
            nc.gpsimd.dma_start(sink, SINK.ap())
            load_wv(0)
            nc.gpsimd.dma_start(masks, MASKS.ap())
            nc.gpsimd.dma_start(ones, ONES.ap())
            load_qk(2)
            load_qk(3)
            # deferred: bias only needed at the out-projection epilogue
            nc.scalar.dma_start(biasb, BIASB.ap())

            emit_proj_interleaved(0)
            emit_proj_interleaved(1)
            # keep the PE clock ramped while waiting for the V weights
            for i in range(22, 40):
                psw = ps_sc.tile([P, 256], F32, tag="sc", name=f"warm{i}")
                nc.tensor.matmul(
                    psw, warm[:, 0:P], warm[:, 256:512], start=True, stop=True
                )
            emit_vproj(0)
            load_wv(1)

            # ---- steady-state: attention(h) interleaved with projection(h+2)
            for h in range(HEADS):
                if h + 4 < HEADS:
                    load_qk(h + 4)
                emit_scores_half(h, 0)
                if h + 2 < HEADS:
                    emit_qproj(h + 2)
                elif h == 14:
                    # tail filler: out-proj (eo0,tt0) for finished heads keeps
                    # the PE busy while heads 14/15's exp chains run
                    pso00 = ps_big.tile([P, 512], F32, tag="big", name="pso0_0")
                    for hd in range(8):
                        nc.tensor.matmul(
                            pso00, out_norm[:, hd, 0:P], wo_tiles[0][:, hd, :],
                            start=(hd == 0), stop=False,
                        )
                elif h == 15:
                    nc.tensor.matmul(
                        pso00, out_norm[:, 14, 0:P], wo_tiles[0][:, 14, :],
                        start=False, stop=False,
                    )
                emit_scores_half(h, 1)
                emit_pv_half(h, 0)
                if h + 2 < HEADS:
                    emit_kproj(h + 2)
                elif h == 14:
                    for hd in range(8, 14):
                        nc.tensor.matmul(
                            pso00, out_norm[:, hd, 0:P], wo_tiles[0][:, hd, :],
                            start=False, stop=False,
                        )
                emit_pv_half(h, 1)
                emit_recip_norm(h)
                if h + 2 < HEADS and (h + 2) % GH == 0:
                    g = (h + 2) // GH
                    emit_vproj(g)
                    if g + 1 < GROUPS:
                        load_wv(g + 1)
                if h == 12:
                    load_wo(0)
                if h == 13:
                    load_wo(1)

            # ---- out projection: OUT[t, e] = out_norm^T . WO + bias ----
            def finish_out(pso, eo, tt):
                o_sb = outsb.tile([P, 512], F32, tag="osb", name=f"o{eo}_{tt}")
                nc.vector.tensor_add(
                    o_sb, pso, biasb[:, eo * 512 : (eo + 1) * 512]
                )
                nc.sync.dma_start(
                    OUT.ap()[tt * P : (tt + 1) * P, eo * 512 : (eo + 1) * 512],
                    o_sb,
                )

            nc.tensor.matmul(
                pso00,
                out_norm[:, 15, 0:P],
                wo_tiles[0][:, 15, :],
                start=False,
                stop=True,
            )
            finish_out(pso00, 0, 0)
            for eo in range(4):
                if eo + 2 < 4:
                    load_wo(eo + 2)
                wo = wo_tiles[eo]
                for tt in range(4):
                    if eo == 0 and tt == 0:
                        continue
                    pso = ps_big.tile([P, 512], F32, tag="big", name=f"pso{eo}_{tt}")
                    if eo == 3 and tt == 3:
                        # split the final tile in two halves so the first
                        # half's bias-add + store overlap the second half
                        for half in range(2):
                            hs = half * 256
                            for hd in range(HEADS):
                                nc.tensor.matmul(
                                    pso[:, hs : hs + 256],
                                    out_norm[:, hd, tt * P : (tt + 1) * P],
                                    wo[:, hd, hs : hs + 256],
                                    start=(hd == 0),
                                    stop=(hd == HEADS - 1),
                                )
                            o_sb = outsb.tile(
                                [P, 256], F32, tag="osbh", name=f"oh{half}"
                            )
                            nc.vector.tensor_add(
                                o_sb, pso[:, hs : hs + 256],
                                biasb[:, eo * 512 + hs : eo * 512 + hs + 256],
                            )
                            nc.sync.dma_start(
                                OUT.ap()[
                                    tt * P : (tt + 1) * P,
                                    eo * 512 + hs : eo * 512 + hs + 256,
                                ],
                                o_sb,
                            )
                        continue
                    for hd in range(HEADS):
                        nc.tensor.matmul(
                            pso,
                            out_norm[:, hd, tt * P : (tt + 1) * P],
                            wo[:, hd, :],
                            start=(hd == 0),
                            stop=(hd == HEADS - 1),
                        )
                    finish_out(pso, eo, tt)

    if legalize:
        _legalize_single_wait(nc)
    return nc


def _quad_perm():
    """Feature permutation: slot 32q+j holds feature 2*(16q + j%16) for j<16
    (pair-even) and 2*(16q + j-16)+1 for j>=16 (pair-odd), so rotate_half is
    a 16-partition swap within each 32-partition quadrant."""
    perm = np.zeros(HD, dtype=np.int64)
    for q in range(4):
        for j in range(32):
            pi = 16 * q + (j % 16)
            perm[32 * q + j] = 2 * pi + (0 if j < 16 else 1)
    return perm


def _rope_tables(pos, scale):
    """Feature-major [128, len(pos)] cos / sin' tables in quadrant-perm order.

    cos'[i,t] = cos(pos_t * invf[pair(i)]); sin' carries -sin on even slots
    (j%32 < 16) and +sin on odd slots.
    """
    inv_freq = 1.0 / (THETA ** (np.arange(0, HD, 2, dtype=np.float64) / HD))  # [64]
    pair_idx = np.zeros(HD, dtype=np.int64)
    sign = np.zeros(HD, dtype=np.float64)
    for q in range(4):
        for j in range(32):
            pair_idx[32 * q + j] = 16 * q + (j % 16)
            sign[32 * q + j] = -1.0 if j < 16 else 1.0
    ang = pos[None, :] * inv_freq[pair_idx][:, None]  # [128, len]
    cos_t = np.cos(ang) * scale
    sin_t = np.sin(ang) * sign[:, None] * scale
    return cos_t.astype(np.float32), sin_t.astype(np.float32)


def _band_masks(start):
    """[P, 4, 256] multiplicative masks (bf16).

    Element (kp, m, qf): role m in {R1 pair0, R2, R3, R1 pair1};
    local key j = c*128 + kp, local query r = qs + qf;
    valid iff r <= j <= r + 128 and (global key) start - 128 + j >= 0.
    """
    out = np.zeros((4, P, 256), dtype=np.float32)
    roles = [(0, 0), (1, 0), (2, 0), (2, 256)]  # (chunk c, query offset qs)
    for m, (c, qs) in enumerate(roles):
        kp = np.arange(P)[:, None]
        qf = np.arange(256)[None, :]
        j = c * P + kp
        r = qs + qf
        valid = (r <= j) & (j <= r + WINDOW) & (start - WINDOW + j >= 0)
        out[m] = valid.astype(np.float32)
    return np.ascontiguousarray(out.transpose(1, 0, 2)).astype(ml_dtypes.bfloat16)


_CACHED = {}
LAST_RESULT = {}


def prepare_in_maps(x, W_qkv, W_out, b_out):
    x = np.asarray(x, dtype=np.float32)
    W_qkv = np.asarray(W_qkv, dtype=np.float32)
    W_out = np.asarray(W_out, dtype=np.float32)
    b_out = np.asarray(b_out, dtype=np.float32)
    bf16 = ml_dtypes.bfloat16

    # host-side weight layout prep
    perm = _quad_perm()
    w4 = W_qkv.reshape(EMBED, HEADS, HD, 3)
    # [h, e, d] -> [h, p, ec, d] partition-major contiguous (perm'd d)
    WQa = w4[..., 0].transpose(1, 0, 2)[:, :, perm].reshape(HEADS, EC, P, HD)
    WQa = np.ascontiguousarray(WQa.transpose(0, 2, 1, 3)).astype(bf16)
    WKa = w4[..., 1].transpose(1, 0, 2)[:, :, perm].reshape(HEADS, EC, P, HD)
    WKa = np.ascontiguousarray(WKa.transpose(0, 2, 1, 3)).astype(bf16)
    # [e, f] -> [g, p, ec, 512]
    WVa = w4[..., 2].reshape(EC, P, 4, 512)
    WVa = np.ascontiguousarray(WVa.transpose(2, 1, 0, 3)).astype(bf16)
    WOa = W_out.reshape(EC, P, 4, 512)
    WOa = np.ascontiguousarray(WOa.transpose(2, 1, 0, 3)).astype(bf16)
    BB = np.ascontiguousarray(
        np.broadcast_to(b_out.reshape(1, EMBED), (P, EMBED))
    ).astype(bf16)
    ones = np.ones((P, P), dtype=np.float32).astype(bf16)

    in_maps = []
    for core in range(NCORES):
        b = core // 4
        start = (core % 4) * S
        # x^T with halo, zero-padded at the left for chunk 0
        xt = np.zeros((EMBED, T), dtype=np.float32)
        lo = start - WINDOW
        src = x[b, max(lo, 0) : start + S, :]  # [<=640, e]
        xt[:, T - src.shape[0] :] = src.T
        xt = np.ascontiguousarray(xt.reshape(EC, P, T)).astype(bf16)
        # rope tables: key positions lo..start+512 (query slice starts at 128)
        kpos = np.maximum(np.arange(lo, start + S, dtype=np.float64), 0.0)
        ck, sk = _rope_tables(kpos, 1.0)
        in_maps.append(
            {
                "XT": xt,
                "WQ": WQa,
                "WK": WKa,
                "WV": WVa,
                "WO": WOa,
                "BIASB": BB,
                "COSK": ck.astype(bf16),
                "SINK": sk.astype(bf16),
                "MASKS": _band_masks(start),
                "ONES": ones,
            }
        )
    return in_maps


def kernel(x, W_qkv, W_out, b_out):
    in_maps = prepare_in_maps(x, W_qkv, W_out, b_out)

    if "nc" not in _CACHED:
        _CACHED["nc"] = build_bass()
    nc = _CACHED["nc"]

    res = run_bass_kernel_spmd(nc, in_maps, core_ids=list(range(NCORES)))
    LAST_RESULT["res"] = res

    out = np.empty((B, L, EMBED), dtype=np.float32)
    for core in range(NCORES):
        b = core // 4
        start = (core % 4) * S
        out[b, start : start + S, :] = res.results[core]["OUT"]
    return out


# revision 46
# speedup vs baseline: 1.0014x; 1.0014x over previous
"""Sliding-window (banded) multi-head self-attention on 8 trn2 NeuronCores.

Sequence-parallel sharding: batch b, 2048 tokens -> 4 chunks of 512 queries;
core c handles batch c//4, chunk c%4.  Each core receives x^T for its 512
tokens plus a 128-token halo (zero-padded for chunk 0), computes
qkv projection + RoPE + banded attention (window 129) + out projection for
its rows, and returns [512, 2048].  No cross-core communication.

v2 (vs. baseline): everything bf16 (halves DMA bytes and PE weight-load
time), single large DMA per weight tensor, per-ec x tiles for fine-grained
startup deps, RoPE pairs laid out 16-apart within each 32-partition quadrant
so rotate_half is ONE DVE stream_shuffle (no SBUF->SBUF DMA), mask-multiply
moved to GpSimd, and a lookahead-2 software pipeline that interleaves head
h's attention matmuls with head h+2's projection matmuls so the PE never
waits on the RoPE/exp chains.  Warm-up dummy matmuls keep the PE clock
ramped through the DMA-bound startup; the first out-projection tile is
emitted early as tail filler while heads 14/15's exp chains drain.

Layout choices (all matmuls contract over the partition dim):
  - x^T resident in SBUF as 16 tiles [128, 640(tok)] bf16 (one per e-chunk)
  - Q^T/K^T per head feature-major [128(d), tok] from PSUM; RoPE pairs
    de-interleaved host-side per 32-partition quadrant (16 evens, 16 odds)
    so rotate_half is stream_shuffle(mask=[16..31,0..15]); the 1/sqrt(d)
    scale is folded into the exp activation's scalar scale.
  - V token-major [128(tok), d] (natural for PV lhsT).
  - scores^T per k-chunk as [128(k), 256(q)] bf16 matmuls; exp on ACT;
    0/1 band-mask multiply on GpSimd; PV + replicated-ones rowsum matmuls
    accumulate into one PSUM bank; normalize with reciprocal + multiply.
  - out projection accumulates 16 hd-chunks; bias added via a DVE
    tensor_add against a host-broadcast bias tile during the PSUM drain.
"""

import math
import numpy as np
import ml_dtypes

import concourse.bass as bass
import concourse.tile as tile
from concourse import mybir
from concourse.bass_utils import run_bass_kernel_spmd
from concourse.vector_clock import ScopedClock, VectorClock


def _legalize_single_wait(nc):
    """This walrus build accepts only ONE sync-wait per lowered command
    ("Too many sync wait commands").  Move all but the last wait of every
    instruction onto single-wait NoOps prepended on the same engine: engines
    are in-order, so stalling on the NoOps is equivalent.  Engine-issued DMAs
    are gated the same way (descriptor push happens in program order)."""
    nid = [0]
    for f in nc.m.functions:
        for blk in f.blocks:
            out = []
            changed = False
            for inst in blk.instructions:
                si = inst.sync_info
                waits = list(si.on_wait) if si and si.on_wait else []
                if len(waits) > 1:
                    changed = True
                    for w in waits[:-1]:
                        nop = mybir.InstNoOp(name=f"waitnop-{nid[0]}", ins=[], outs=[])
                        nid[0] += 1
                        nop.engine = inst.engine
                        nop.sync_info = mybir.SyncInfo(on_wait=[w], on_update=[])
                        out.append(nop)
                    inst.sync_info = mybir.SyncInfo(
                        on_wait=[waits[-1]], on_update=list(si.on_update or [])
                    )
                out.append(inst)
            if changed:
                blk.instructions = out
    return nc


def _install_drain_split_patch():
    """Split TileContext's closing drain into single-wait drains: walrus's
    CTRL_NO command rejects the catch-all drain ("Too many sync waits")."""
    if getattr(tile.TileContext, "_drain_split_patched", False):
        return

    def _patched(self, tick_clock, wait_clock):
        gvc = tick_clock.global_clock  # VectorClock over the 27 procs
        n = len(gvc)
        procs = [i for i in range(n) if gvc[i] > 0]
        for pi in procs:
            vc = VectorClock([gvc[i] if i == pi else 0 for i in range(n)])
            d = self.nc.sync.drain()
            wait_clock.add_sem_waits(d.ins, ScopedClock({None: vc}))
        self.nc.all_engine_barrier()
        assert self.sems is not None
        popped = self.nc._tile_sem_poison_stack.pop()
        assert popped is self._sem_poison
        self.nc.clear_and_free_semaphores(list(self.sems.allocated().values()))
        self.nc.all_engine_barrier()

    tile.TileContext._drain_and_barrier = _patched
    tile.TileContext._drain_split_patched = True


_install_drain_split_patch()

EMBED = 2048
HEADS = 16
HD = 128
WINDOW = 128
THETA = 10000.0
B = 2
L = 2048
S = 512            # queries per core
T = S + WINDOW     # k/v tokens per core (incl halo)
NCORES = 8
P = 128
F32 = mybir.dt.float32
BF16 = mybir.dt.bfloat16

EC = EMBED // P    # 16 e-chunks
GROUPS = 4         # head groups of 4 (for V projection at N=512)
GH = HEADS // GROUPS
HT = T // 2        # 320

# rotate_half as a within-quadrant shuffle: out[32s+i] = in[32s+mask[i]]
SHUF = list(range(16, 32)) + list(range(0, 16))


def build_bass(legalize=True):
    nc = bass.Bass("TRN2", target_bir_lowering=False, debug=False)

    XT = nc.dram_tensor("XT", [EC, P, T], BF16, kind="ExternalInput")
    WQ = nc.dram_tensor("WQ", [HEADS, P, EC, HD], BF16, kind="ExternalInput")
    WK = nc.dram_tensor("WK", [HEADS, P, EC, HD], BF16, kind="ExternalInput")
    WV = nc.dram_tensor("WV", [GROUPS, P, EC, 512], BF16, kind="ExternalInput")
    WO = nc.dram_tensor("WO", [4, P, EC, 512], BF16, kind="ExternalInput")
    BIASB = nc.dram_tensor("BIASB", [P, EMBED], BF16, kind="ExternalInput")
    COSK = nc.dram_tensor("COSK", [P, T], BF16, kind="ExternalInput")
    SINK = nc.dram_tensor("SINK", [P, T], BF16, kind="ExternalInput")
    MASKS = nc.dram_tensor("MASKS", [P, 4, 256], BF16, kind="ExternalInput")
    ONES = nc.dram_tensor("ONES", [P, P], BF16, kind="ExternalInput")
    OUT = nc.dram_tensor("OUT", [S, EMBED], F32, kind="ExternalOutput")

    with tile.TileContext(nc) as tc:
        with (
            tc.tile_pool(name="persist", bufs=1) as persist,
            tc.tile_pool(name="wqk", bufs=8) as wqk,
            tc.tile_pool(name="wvp", bufs=2) as wvp,
            tc.tile_pool(name="wop", bufs=2) as wop,
            tc.tile_pool(name="rope", bufs=8) as rope,
            tc.tile_pool(name="qk", bufs=8) as qk,
            tc.tile_pool(name="vsb", bufs=10) as vsb,
            tc.tile_pool(name="etp", bufs=6) as etp,
            tc.tile_pool(name="denp", bufs=4) as denp,
            tc.tile_pool(name="outsb", bufs=2) as outsb,
            tc.tile_pool(name="ps_big", bufs=3, space="PSUM") as ps_big,
            tc.tile_pool(name="ps_k", bufs=2, space="PSUM") as ps_k,
            tc.tile_pool(name="ps_sc", bufs=3, space="PSUM") as ps_sc,
        ):
            # ---- persistent tiles ----
            xt = [
                persist.tile([P, T], BF16, tag=f"xt{ec}", name=f"xt{ec}")
                for ec in range(EC)
            ]
            cosk = persist.tile([P, T], BF16, tag="cosk")
            sink = persist.tile([P, T], BF16, tag="sink")
            cosq = cosk[:, WINDOW:T]
            sinq = sink[:, WINDOW:T]
            masks = persist.tile([P, 4, 256], BF16, tag="masks")
            ones = persist.tile([P, P], BF16, tag="ones")
            biasb = persist.tile([P, EMBED], BF16, tag="biasb")
            out_norm = persist.tile([P, HEADS, S], BF16, tag="out_norm")

            wq_tiles, wk_tiles, wv_tiles, wo_tiles = {}, {}, {}, {}
            q_tiles, k_tiles, v_groups, ets, pocs = {}, {}, {}, {}, {}

            def load_qk(h, eng=None):
                eng = eng or nc.scalar
                wq = wqk.tile([P, EC, HD], BF16, tag="wqk", name=f"wq{h}")
                eng.dma_start(wq, WQ.ap()[h])
                wk = wqk.tile([P, EC, HD], BF16, tag="wqk", name=f"wk{h}")
                eng.dma_start(wk, WK.ap()[h])
                wq_tiles[h] = wq
                wk_tiles[h] = wk

            def load_wv(g):
                wv = wvp.tile([P, EC, 512], BF16, tag="wv", name=f"wv{g}")
                nc.sync.dma_start(wv, WV.ap()[g])
                wv_tiles[g] = wv

            def load_wo(eo):
                wo = wop.tile([P, EC, 512], BF16, tag="wo", name=f"wo{eo}")
                nc.sync.dma_start(wo, WO.ap()[eo])
                wo_tiles[eo] = wo

            def emit_qproj(h):
                wq = wq_tiles[h]
                psq = ps_big.tile([P, S], F32, tag="big", name=f"psq{h}")
                for ec in range(EC):
                    nc.tensor.matmul(
                        psq,
                        wq[:, ec, :],
                        xt[ec][:, WINDOW:T],
                        start=(ec == 0),
                        stop=(ec == EC - 1),
                    )
                qsw = rope.tile([P, S], F32, tag="rope", name=f"qsw{h}")
                nc.vector.stream_shuffle(qsw, psq, SHUF)
                nc.gpsimd.tensor_mul(qsw, qsw, sinq)
                qc = rope.tile([P, S], F32, tag="rope", name=f"qc{h}")
                nc.vector.tensor_mul(qc, psq, cosq)
                q_sb = qk.tile([P, S], BF16, tag="qk", name=f"q{h}")
                nc.vector.tensor_add(q_sb, qc, qsw)
                q_tiles[h] = q_sb

            def emit_proj_interleaved(h):
                # startup variant: Q/K matmuls in 4-ec blocks so the PE
                # tracks the streaming x arrival instead of stalling per pass
                wq, wk = wq_tiles[h], wk_tiles[h]
                psq = ps_big.tile([P, S], F32, tag="big", name=f"psq{h}")
                psk1 = ps_k.tile([P, HT], F32, tag="k", name=f"psk1_{h}")
                psk2 = ps_k.tile([P, HT], F32, tag="k", name=f"psk2_{h}")
                for blk in range(4):
                    for ec in range(4 * blk, 4 * blk + 4):
                        nc.tensor.matmul(
                            psq, wq[:, ec, :], xt[ec][:, WINDOW:T],
                            start=(ec == 0), stop=(ec == EC - 1),
                        )
                    for ec in range(4 * blk, 4 * blk + 4):
                        nc.tensor.matmul(
                            psk1, wk[:, ec, :], xt[ec][:, 0:HT],
                            start=(ec == 0), stop=(ec == EC - 1),
                        )
                    for ec in range(4 * blk, 4 * blk + 4):
                        nc.tensor.matmul(
                            psk2, wk[:, ec, :], xt[ec][:, HT:T],
                            start=(ec == 0), stop=(ec == EC - 1),
                        )
                qsw = rope.tile([P, S], F32, tag="rope", name=f"qsw{h}")
                nc.vector.stream_shuffle(qsw, psq, SHUF)
                nc.gpsimd.tensor_mul(qsw, qsw, sinq)
                qc = rope.tile([P, S], F32, tag="rope", name=f"qc{h}")
                nc.vector.tensor_mul(qc, psq, cosq)
                q_sb = qk.tile([P, S], BF16, tag="qk", name=f"q{h}")
                nc.vector.tensor_add(q_sb, qc, qsw)
                q_tiles[h] = q_sb
                ksw = rope.tile([P, T], F32, tag="rope", name=f"ksw{h}")
                nc.vector.stream_shuffle(ksw[:, 0:HT], psk1, SHUF)
                nc.vector.stream_shuffle(ksw[:, HT:T], psk2, SHUF)
                nc.gpsimd.tensor_mul(ksw, ksw, sink)
                kc = rope.tile([P, T], F32, tag="rope", name=f"kc{h}")
                nc.vector.tensor_mul(kc[:, 0:HT], psk1, cosk[:, 0:HT])
                nc.vector.tensor_mul(kc[:, HT:T], psk2, cosk[:, HT:T])
                k_sb = qk.tile([P, T], BF16, tag="qk", name=f"k{h}")
                nc.vector.tensor_add(k_sb, kc, ksw)
                k_tiles[h] = k_sb

            def emit_kproj(h):
                wk = wk_tiles[h]
                psk1 = ps_k.tile([P, HT], F32, tag="k", name=f"psk1_{h}")
                psk2 = ps_k.tile([P, HT], F32, tag="k", name=f"psk2_{h}")
                for ec in range(EC):
                    nc.tensor.matmul(
                        psk1,
                        wk[:, ec, :],
                        xt[ec][:, 0:HT],
                        start=(ec == 0),
                        stop=(ec == EC - 1),
                    )
                for ec in range(EC):
                    nc.tensor.matmul(
                        psk2,
                        wk[:, ec, :],
                        xt[ec][:, HT:T],
                        start=(ec == 0),
                        stop=(ec == EC - 1),
                    )
                ksw = rope.tile([P, T], F32, tag="rope", name=f"ksw{h}")
                nc.vector.stream_shuffle(ksw[:, 0:HT], psk1, SHUF)
                nc.vector.stream_shuffle(ksw[:, HT:T], psk2, SHUF)
                nc.gpsimd.tensor_mul(ksw, ksw, sink)
                kc = rope.tile([P, T], F32, tag="rope", name=f"kc{h}")
                nc.vector.tensor_mul(kc[:, 0:HT], psk1, cosk[:, 0:HT])
                nc.vector.tensor_mul(kc[:, HT:T], psk2, cosk[:, HT:T])
                k_sb = qk.tile([P, T], BF16, tag="qk", name=f"k{h}")
                nc.vector.tensor_add(k_sb, kc, ksw)
                k_tiles[h] = k_sb

            def emit_vproj(g):
                wv = wv_tiles[g]
                vts = []
                for tt in range(T // P):
                    psv = ps_big.tile([P, 512], F32, tag="big", name=f"psv{g}_{tt}")
                    for ec in range(EC):
                        nc.tensor.matmul(
                            psv,
                            xt[ec][:, tt * P : (tt + 1) * P],
                            wv[:, ec, :],
                            start=(ec == 0),
                            stop=(ec == EC - 1),
                        )
                    v_t = vsb.tile([P, 512], BF16, tag="v", name=f"v{g}_{tt}")
                    nc.scalar.copy(v_t, psv)
                    vts.append(v_t)
                v_groups[g] = vts

            def emit_scores_half(h, p):
                qs = p * 256
                for j in range(3):
                    c = 2 * p + j
                    midx = 3 if (j == 0 and p == 1) else j
                    psc = ps_sc.tile([P, 256], F32, tag="sc", name=f"sc{h}_{p}{j}")
                    nc.tensor.matmul(
                        psc,
                        k_tiles[h][:, c * P : (c + 1) * P],
                        q_tiles[h][:, qs : qs + 256],
                        start=True,
                        stop=True,
                    )
                    et = etp.tile([P, 256], BF16, tag="et", name=f"et{h}_{p}{j}")
                    nc.scalar.activation(
                        et, psc, mybir.ActivationFunctionType.Exp,
                        scale=1.0 / math.sqrt(HD),
                    )
                    nc.gpsimd.tensor_mul(et, et, masks[:, midx, :])
                    ets[(h, p, j)] = et

            def emit_pv_half(h, p):
                hh = h % GH
                vts = v_groups[h // GH]
                poc = ps_big.tile([P, 512], F32, tag="big", name=f"poc{h}_{p}")
                for j in range(3):
                    c = 2 * p + j
                    et = ets.pop((h, p, j))
                    nc.tensor.matmul(
                        poc[:, 0:256],
                        vts[c][:, hh * HD : (hh + 1) * HD],
                        et,
                        start=(j == 0),
                        stop=False,
                    )
                    nc.tensor.matmul(
                        poc[:, 256:512],
                        ones,
                        et,
                        start=False,
                        stop=(j == 2),
                    )
                pocs[(h, p)] = poc

            def emit_recip_norm(h):
                for p in range(2):
                    poc = pocs.pop((h, p))
                    rc = denp.tile([P, 256], F32, tag="rc", name=f"rc{h}_{p}")
                    nc.vector.reciprocal(rc, poc[:, 256:512])
                    nc.vector.tensor_mul(
                        out_norm[:, h, p * 256 : (p + 1) * 256], poc[:, 0:256], rc
                    )

            # ---- prologue: loads + heads 0,1 projection + V group 0 ----
            # p-state warmup: dummy matmuls on memset data keep the PE busy
            # from ~8us so the clock is ramped when real work arrives ~12us
            warm = persist.tile([P, 512], BF16, tag="warm")
            nc.gpsimd.memset(warm, 0.0)
            for i in range(22):
                psw = ps_sc.tile([P, 256], F32, tag="sc", name=f"warm{i}")
                nc.tensor.matmul(
                    psw, warm[:, 0:P], warm[:, 256:512], start=True, stop=True
                )

            load_qk(0)
            for ec in range(EC // 2):
                nc.sync.dma_start(xt[ec], XT.ap()[ec])
            for ec in range(EC // 2, EC):
                nc.scalar.dma_start(xt[ec], XT.ap()[ec])
            load_qk(1)
            nc.gpsimd.dma_start(cosk, COSK.ap())
            nc.gpsimd.dma_start(sink, SINK.ap())
            load_wv(0)
            nc.gpsimd.dma_start(masks, MASKS.ap())
            nc.gpsimd.dma_start(ones, ONES.ap())
            load_qk(2)
            load_qk(3)
            # deferred: bias only needed at the out-projection epilogue
            nc.scalar.dma_start(biasb, BIASB.ap())

            emit_proj_interleaved(0)
            emit_proj_interleaved(1)
            # keep the PE clock ramped while waiting for the V weights
            for i in range(22, 40):
                psw = ps_sc.tile([P, 256], F32, tag="sc", name=f"warm{i}")
                nc.tensor.matmul(
                    psw, warm[:, 0:P], warm[:, 256:512], start=True, stop=True
                )
            emit_vproj(0)
            load_wv(1)

            # ---- steady-state: attention(h) interleaved with projection(h+2)
            for h in range(HEADS):
                if h + 4 < HEADS:
                    load_qk(h + 4)
                emit_scores_half(h, 0)
                if h + 2 < HEADS:
                    emit_qproj(h + 2)
                elif h == 14:
                    # tail filler: out-proj (eo0,tt0) for finished heads keeps
                    # the PE busy while heads 14/15's exp chains run
                    pso00 = ps_big.tile([P, 512], F32, tag="big", name="pso0_0")
                    for hd in range(8):
                        nc.tensor.matmul(
                            pso00, out_norm[:, hd, 0:P], wo_tiles[0][:, hd, :],
                            start=(hd == 0), stop=False,
                        )
                elif h == 15:
                    nc.tensor.matmul(
                        pso00, out_norm[:, 14, 0:P], wo_tiles[0][:, 14, :],
                        start=False, stop=False,
                    )
                emit_scores_half(h, 1)
                emit_pv_half(h, 0)
                if h + 2 < HEADS:
                    emit_kproj(h + 2)
                elif h == 14:
                    for hd in range(8, 14):
                        nc.tensor.matmul(
                            pso00, out_norm[:, hd, 0:P], wo_tiles[0][:, hd, :],
                            start=False, stop=False,
                        )
                emit_pv_half(h, 1)
                emit_recip_norm(h)
                if h + 2 < HEADS and (h + 2) % GH == 0:
                    g = (h + 2) // GH
                    emit_vproj(g)
                    if g + 1 < GROUPS:
                        load_wv(g + 1)
                if h == 12:
                    load_wo(0)
                if h == 13:
                    load_wo(1)

            # ---- out projection: OUT[t, e] = out_norm^T . WO + bias ----
            def finish_out(pso, eo, tt):
                o_sb = outsb.tile([P, 512], F32, tag="osb", name=f"o{eo}_{tt}")
                nc.vector.tensor_add(
                    o_sb, pso, biasb[:, eo * 512 : (eo + 1) * 512]
                )
                nc.sync.dma_start(
                    OUT.ap()[tt * P : (tt + 1) * P, eo * 512 : (eo + 1) * 512],
                    o_sb,
                )

            nc.tensor.matmul(
                pso00,
                out_norm[:, 15, 0:P],
                wo_tiles[0][:, 15, :],
                start=False,
                stop=True,
            )
            finish_out(pso00, 0, 0)
            for eo in range(4):
                if eo + 2 < 4:
                    load_wo(eo + 2)
                wo = wo_tiles[eo]
                for tt in range(4):
                    if eo == 0 and tt == 0:
                        continue
                    pso = ps_big.tile([P, 512], F32, tag="big", name=f"pso{eo}_{tt}")
                    if eo == 3 and tt == 3:
                        # split the final tile in two halves so the first
                        # half's bias-add + store overlap the second half
                        for half in range(2):
                            hs = half * 256
                            for hd in range(HEADS):
                                nc.tensor.matmul(
                                    pso[:, hs : hs + 256],
                                    out_norm[:, hd, tt * P : (tt + 1) * P],
                                    wo[:, hd, hs : hs + 256],
                                    start=(hd == 0),
                                    stop=(hd == HEADS - 1),
                                )
                            o_sb = outsb.tile(
                                [P, 256], F32, tag="osbh", name=f"oh{half}"
                            )
                            nc.vector.tensor_add(
                                o_sb, pso[:, hs : hs + 256],
                                biasb[:, eo * 512 + hs : eo * 512 + hs + 256],
                            )
                            nc.sync.dma_start(
                                OUT.ap()[
                                    tt * P : (tt + 1) * P,
                                    eo * 512 + hs : eo * 512 + hs + 256,
                                ],
                                o_sb,
                            )
                        continue
                    for hd in range(HEADS):
                        nc.tensor.matmul(
                            pso,
                            out_norm[:, hd, tt * P : (tt + 1) * P],
                            wo[:, hd, :],
                            start=(hd == 0),
                            stop=(hd == HEADS - 1),
                        )
                    finish_out(pso, eo, tt)

    if legalize:
        _legalize_single_wait(nc)
    return nc


def _quad_perm():
    """Feature permutation: slot 32q+j holds feature 2*(16q + j%16) for j<16
    (pair-even) and 2*(16q + j-16)+1 for j>=16 (pair-odd), so rotate_half is
    a 16-partition swap within each 32-partition quadrant."""
    perm = np.zeros(HD, dtype=np.int64)
    for q in range(4):
        for j in range(32):
            pi = 16 * q + (j % 16)
            perm[32 * q + j] = 2 * pi + (0 if j < 16 else 1)
    return perm


def _rope_tables(pos, scale):
    """Feature-major [128, len(pos)] cos / sin' tables in quadrant-perm order.

    cos'[i,t] = cos(pos_t * invf[pair(i)]); sin' carries -sin on even slots
    (j%32 < 16) and +sin on odd slots.
    """
    inv_freq = 1.0 / (THETA ** (np.arange(0, HD, 2, dtype=np.float64) / HD))  # [64]
    pair_idx = np.zeros(HD, dtype=np.int64)
    sign = np.zeros(HD, dtype=np.float64)
    for q in range(4):
        for j in range(32):
            pair_idx[32 * q + j] = 16 * q + (j % 16)
            sign[32 * q + j] = -1.0 if j < 16 else 1.0
    ang = pos[None, :] * inv_freq[pair_idx][:, None]  # [128, len]
    cos_t = np.cos(ang) * scale
    sin_t = np.sin(ang) * sign[:, None] * scale
    return cos_t.astype(np.float32), sin_t.astype(np.float32)


def _band_masks(start):
    """[P, 4, 256] multiplicative masks (bf16).

    Element (kp, m, qf): role m in {R1 pair0, R2, R3, R1 pair1};
    local key j = c*128 + kp, local query r = qs + qf;
    valid iff r <= j <= r + 128 and (global key) start - 128 + j >= 0.
    """
    out = np.zeros((4, P, 256), dtype=np.float32)
    roles = [(0, 0), (1, 0), (2, 0), (2, 256)]  # (chunk c, query offset qs)
    for m, (c, qs) in enumerate(roles):
        kp = np.arange(P)[:, None]
        qf = np.arange(256)[None, :]
        j = c * P + kp
        r = qs + qf
        valid = (r <= j) & (j <= r + WINDOW) & (start - WINDOW + j >= 0)
        out[m] = valid.astype(np.float32)
    return np.ascontiguousarray(out.transpose(1, 0, 2)).astype(ml_dtypes.bfloat16)


_CACHED = {}
LAST_RESULT = {}


def prepare_in_maps(x, W_qkv, W_out, b_out):
    x = np.asarray(x, dtype=np.float32)
    W_qkv = np.asarray(W_qkv, dtype=np.float32)
    W_out = np.asarray(W_out, dtype=np.float32)
    b_out = np.asarray(b_out, dtype=np.float32)
    bf16 = ml_dtypes.bfloat16

    # host-side weight layout prep
    perm = _quad_perm()
    w4 = W_qkv.reshape(EMBED, HEADS, HD, 3)
    # [h, e, d] -> [h, p, ec, d] partition-major contiguous (perm'd d)
    WQa = w4[..., 0].transpose(1, 0, 2)[:, :, perm].reshape(HEADS, EC, P, HD)
    WQa = np.ascontiguousarray(WQa.transpose(0, 2, 1, 3)).astype(bf16)
    WKa = w4[..., 1].transpose(1, 0, 2)[:, :, perm].reshape(HEADS, EC, P, HD)
    WKa = np.ascontiguousarray(WKa.transpose(0, 2, 1, 3)).astype(bf16)
    # [e, f] -> [g, p, ec, 512]
    WVa = w4[..., 2].reshape(EC, P, 4, 512)
    WVa = np.ascontiguousarray(WVa.transpose(2, 1, 0, 3)).astype(bf16)
    WOa = W_out.reshape(EC, P, 4, 512)
    WOa = np.ascontiguousarray(WOa.transpose(2, 1, 0, 3)).astype(bf16)
    BB = np.ascontiguousarray(
        np.broadcast_to(b_out.reshape(1, EMBED), (P, EMBED))
    ).astype(bf16)
    ones = np.ones((P, P), dtype=np.float32).astype(bf16)

    in_maps = []
    for core in range(NCORES):
        b = core // 4
        start = (core % 4) * S
        # x^T with halo, zero-padded at the left for chunk 0
        xt = np.zeros((EMBED, T), dtype=np.float32)
        lo = start - WINDOW
        src = x[b, max(lo, 0) : start + S, :]  # [<=640, e]
        xt[:, T - src.shape[0] :] = src.T
        xt = np.ascontiguousarray(xt.reshape(EC, P, T)).astype(bf16)
        # rope tables: key positions lo..start+512 (query slice starts at 128)
        kpos = np.maximum(np.arange(lo, start + S, dtype=np.float64), 0.0)
        ck, sk = _rope_tables(kpos, 1.0)
        in_maps.append(
            {
                "XT": xt,
                "WQ": WQa,
                "WK": WKa,
                "WV": WVa,
                "WO": WOa,
                "BIASB": BB,
                "COSK": ck.astype(bf16),
                "SINK": sk.astype(bf16),
                "MASKS": _band_masks(start),
                "ONES": ones,
            }
        )
    return in_maps


def kernel(x, W_qkv, W_out, b_out):
    in_maps = prepare_in_maps(x, W_qkv, W_out, b_out)

    if "nc" not in _CACHED:
        _CACHED["nc"] = build_bass()
    nc = _CACHED["nc"]

    res = run_bass_kernel_spmd(nc, in_maps, core_ids=list(range(NCORES)))
    LAST_RESULT["res"] = res

    out = np.empty((B, L, EMBED), dtype=np.float32)
    for core in range(NCORES):
        b = core // 4
        start = (core % 4) * S
        out[b, start : start + S, :] = res.results[core]["OUT"]
    return out
